# revision 12
# baseline (speedup 1.0000x reference)
"""Fused multi-head attention block on 8 TRN2 NeuronCores.

reference: qkv = x@Wqkv+b; q,k rmsnorm'd per head; softmax(q k^T/sqrt(hd)) v; proj.
Shapes: x [2,2048,1024], H=16 heads, hd=64.

Distribution (no collectives): 8 cores = 2 batches x 4 head-groups (4 heads each).
Core c: batch b=c//4, heads 4g..4g+3 (g=c%4). Each core computes the partial
projection output (proj_w row-sharded over its heads) for its batch; the host
sums the 4 partials per batch and adds proj_b.

Per-core pipeline (bf16 matmul operands, f32 PSUM accumulation):
  A) qkv GEMM with OUTPUT channel-major for q,k (wqkv stationary, x^T moving)
     so q^T/k^T need no PE transposes; v computed token-major (x^T stationary)
     straight into the AV stationary layout. rmsnorm per head: sq=(q*w)^2 on
     DVE (w folded into wqkv on host); per-head sum-of-squares via a
     block-diagonal 1/(64 w^2) matmul that REPLICATES the stat across the
     head's 64 partitions; sqrt on ACT; reciprocal_approx_fast on DVE; one
     bf16 multiply applies rstd.
  B) attention per (q-512-chunk, head-pair): the two heads' K=64 score matmuls
     run CONCURRENTLY in separate PE row-groups (partitions 0:64 / 64:128).
     exp(s/8) split between ACT (Exp activation) and DVE (Schraudolph bf16
     bit-trick: s*a+b -> int32, read high half-words as bf16). AV accumulates
     with stationary [v|ones]/[ones|v] giving out^T and the softmax
     denominator in one pass. Epilogue: aT = out^T * recip(denom).
  C) partial projection from aT (interleaved into the next group's kt loop).
"""

from contextlib import ExitStack

import ml_dtypes
import numpy as np

import concourse.bass as bass
import concourse.mybir as mybir
import concourse.tile as tile
from concourse import bacc
from concourse.bass_utils import run_bass_kernel_spmd

B, N, C = 2, 2048, 1024
H, HD = 16, 64
HPC = 4                 # heads per core
NT = N // 128           # 16 k-token tiles
KT8 = C // 128          # 8 contraction tiles for the qkv GEMM
QK = 2 * HPC * HD       # 512 qk channels per core
V = HPC * HD            # 256 v channels per core
EPS = 1e-6
F32 = mybir.dt.float32
BF16 = mybir.dt.bfloat16
I16 = mybir.dt.int16
AF = mybir.ActivationFunctionType
MUL = mybir.AluOpType.mult
ADD = mybir.AluOpType.add

LOG2E = 1.4426950408889634
# exp(s/8) ~= bf16_frombits(int16(s*A16 + B16)): schraudolph with the /8
# softmax scale folded in; B16 centers the sawtooth error (C ~= 5.5/128).
A16 = 128.0 * LOG2E / 8.0
B16 = 16251.0

# which kt tiles' exp goes to DVE (schraudolph) instead of ACT, per 16-kt group
# (none at the tail so the group epilogue isn't queued behind schraudolphs)
DVE_KTS = frozenset((1, 3, 6, 8, 10, 12))


def build_nc():
    nc = bacc.Bacc("TRN2", target_bir_lowering=False, debug=False)

    x_ext = nc.declare_dram_parameter("x", [C, N], BF16, isOutput=False)
    wqkv_ext = nc.declare_dram_parameter("wqkv", [C, QK + V], BF16, isOutput=False)
    bqk_ext = nc.declare_dram_parameter("bqk", [128, 4], F32, isOutput=False)
    bv_ext = nc.declare_dram_parameter("bv", [V], F32, isOutput=False)
    inc_ext = nc.declare_dram_parameter("inc", [128, 2, 128], BF16, isOutput=False)
    wproj_ext = nc.declare_dram_parameter("wproj", [V // 2, 2, C], BF16, isOutput=False)
    out_ext = nc.declare_dram_parameter("out", [N, C], F32, isOutput=True)

    with tile.TileContext(nc) as tc, ExitStack() as ctx:
        singles = ctx.enter_context(tc.tile_pool(name="singles", bufs=1))

        # interleave weight/x chunk loads kt-major so the first GEMM matmuls
        # (which consume kt in order) start after ~2 chunks instead of the
        # full 5.5 MB
        wqkv_sb = singles.tile([128, KT8, QK + V], BF16, tag="wqkv")
        xt_all = singles.tile([128, KT8, N], BF16, tag="xt_all")
        for kt in range(KT8):
            nc.sync.dma_start(
                out=wqkv_sb[:, kt, :], in_=wqkv_ext[kt * 128:(kt + 1) * 128, :]
            )
            nc.scalar.dma_start(
                out=xt_all[:, kt, :], in_=x_ext[kt * 128:(kt + 1) * 128, :])
        wproj_sb = singles.tile([128, 2, C], BF16, tag="wproj")
        nc.sync.dma_start(out=wproj_sb, in_=wproj_ext[:, :, :])
        bqk_sb = singles.tile([128, 4], F32, tag="bqk")
        nc.sync.dma_start(out=bqk_sb, in_=bqk_ext[:, :])
        bv_sb = singles.tile([128, V], F32, tag="bv")
        nc.sync.dma_start(out=bv_sb, in_=bv_ext[:].partition_broadcast(128))
        inc_sb = singles.tile([128, 2, 128], BF16, tag="inc")
        nc.sync.dma_start(out=inc_sb, in_=inc_ext[:, :, :])
        eps_sb = singles.tile([128, 1], F32, tag="eps")
        nc.vector.memset(eps_sb, EPS)
        junk = singles.tile([128, 1], F32, tag="junk")

        # persistent activations (all channel-major)
        qn = singles.tile([128, 2, N], BF16, tag="qn")
        kn = singles.tile([128, 2, N], BF16, tag="kn")
        # AV stationary blocks: even local head: [v | ones]; odd: [ones | v]
        vaug = singles.tile([128, NT, HPC, 128], BF16, tag="vaug")
        nc.gpsimd.memset(vaug, 1.0)
        # normalized attention output; block hb packs heads 2hb,2hb+1
        aT = singles.tile([128, 2, N], BF16, tag="aT")

        # ---------------- phase A: qkv + rmsnorm, no transposes --------------
        with ExitStack() as actx:
            pqk = actx.enter_context(tc.tile_pool(name="pqk", bufs=3, space="PSUM"))
            pv = actx.enter_context(tc.tile_pool(name="pv", bufs=3, space="PSUM"))
            pssq = actx.enter_context(tc.tile_pool(name="pssq", bufs=2, space="PSUM"))
            qsbp = actx.enter_context(tc.tile_pool(name="qsb", bufs=3))
            sqp = actx.enter_context(tc.tile_pool(name="sqp", bufs=2))
            smp = actx.enter_context(tc.tile_pool(name="smp", bufs=2))
            rstdp = actx.enter_context(tc.tile_pool(name="rstd", bufs=2))

            pend = None     # (sq, cb, tb) awaiting the ssq matmul + rstd chain

            def finish_norm(sq, cb, tb):
                tsl = slice(tb * 512, (tb + 1) * 512)
                p_ssq = pssq.tile([128, 512], F32, tag="pssq")
                nc.tensor.matmul(p_ssq, inc_sb[:, cb // 2, :], sq[1],
                                 start=True, stop=True)
                sm = smp.tile([128, 512], F32, tag="sm")
                nc.scalar.activation(sm, p_ssq, AF.Sqrt,
                                     bias=eps_sb[:, 0:1], scale=1.0)
                rstd = rstdp.tile([128, 512], F32, tag="rstd")
                nc.vector.reciprocal_approx_fast(rstd, sm)
                dst = qn if cb < 2 else kn
                nc.vector.tensor_tensor(dst[:, cb % 2, tsl], sq[0], rstd, op=MUL)

            for tb in range(4):
                tsl = slice(tb * 512, (tb + 1) * 512)
                for cb in range(4):
                    p_qk = pqk.tile([128, 512], F32, tag="pqk")
                    for kt in range(KT8):
                        nc.tensor.matmul(
                            p_qk, wqkv_sb[:, kt, cb * 128:(cb + 1) * 128],
                            xt_all[:, kt, tsl],
                            start=(kt == 0), stop=(kt == KT8 - 1),
                        )
                    if pend is not None:
                        finish_norm(*pend)
                    qsb = qsbp.tile([128, 512], BF16, tag="qsb")
                    nc.scalar.activation(qsb, p_qk, AF.Identity,
                                         bias=bqk_sb[:, cb:cb + 1], scale=1.0)
                    sq = sqp.tile([128, 512], BF16, tag="sq")
                    nc.scalar.activation(sq, p_qk, AF.Square,
                                         bias=bqk_sb[:, cb:cb + 1], scale=1.0)
                    pend = ((qsb, sq), cb, tb)

                    # one v token-tile interleaved after each qk block
                    t = tb * 4 + cb
                    ts = slice(t * 128, (t + 1) * 128)
                    p_v = pv.tile([128, V], F32, tag="pv")
                    for kt in range(KT8):
                        nc.tensor.matmul(
                            p_v, xt_all[:, kt, ts], wqkv_sb[:, kt, QK:QK + V],
                            start=(kt == 0), stop=(kt == KT8 - 1),
                        )
                    pv3 = p_v.rearrange("p (h d) -> p h d", d=HD)
                    bv3 = bv_sb.rearrange("p (h d) -> p h d", d=HD)
                    nc.vector.tensor_add(vaug[:, t, 0::2, 0:HD], pv3[:, 0::2, :],
                                         bv3[:, 0::2, :])
                    nc.vector.tensor_add(vaug[:, t, 1::2, HD:128], pv3[:, 1::2, :],
                                         bv3[:, 1::2, :])
            finish_norm(*pend)
            # prefetch the exp table set while the phase-A tail drains
            nc.scalar.activation(junk, eps_sb, AF.Exp, scale=1.0)

        # -------- phase B: attention + projection (proj interleaved) --------
        with ExitStack() as bctx:
            spool = bctx.enter_context(tc.tile_pool(name="ps", bufs=3, space="PSUM"))
            opool = bctx.enter_context(tc.tile_pool(name="po", bufs=1, space="PSUM"))
            ptpool = bctx.enter_context(tc.tile_pool(name="pt", bufs=4))
            rpool = bctx.enter_context(tc.tile_pool(name="rec", bufs=2))
            outpool = bctx.enter_context(tc.tile_pool(name="outsb", bufs=4))

            def emit_scores(cbp, qsl, kt):
                ps = spool.tile([128, 2, 512], F32, tag="ps")
                ksl = slice(kt * 128, (kt + 1) * 128)
                for hh in range(2):
                    rows = slice(hh * 64, (hh + 1) * 64)
                    nc.tensor.matmul(
                        ps[:, hh, :], kn[rows, cbp, ksl],
                        qn[rows, cbp, qsl], start=True, stop=True,
                    )
                return ps

            def emit_proj_tile(t):
                """partial projection for q token tile t (pp from the spool
                ring; evicts split ACT/DVE; DMA overlaps later groups)"""
                ts = slice(t * 128, (t + 1) * 128)
                pp = spool.tile([128, 2, 512], F32, tag="ps")
                for hb in range(2):        # stationary aT block reused over jg
                    for jg in range(2):
                        nc.tensor.matmul(
                            pp[:, jg, :], aT[:, hb, ts],
                            wproj_sb[:, hb, jg * 512:(jg + 1) * 512],
                            start=(hb == 0), stop=(hb == 1),
                        )
                for jg in range(2):
                    outsb = outpool.tile([128, 512], F32, tag="outsb")
                    if jg == 0:
                        nc.scalar.activation(outsb, pp[:, jg, :], AF.Copy)
                    else:
                        nc.vector.tensor_copy(outsb, pp[:, jg, :])
                    nc.sync.dma_start(
                        out=out_ext[ts, jg * 512:(jg + 1) * 512], in_=outsb
                    )

            proj_pend = []
            for qh4 in range(4):           # 512-token chunks of the query axis
                qsl = slice(qh4 * 512, (qh4 + 1) * 512)
                for cbp in range(2):       # head pair (2cbp, 2cbp+1)
                    po = opool.tile([128, 2, 512], F32, tag="po")
                    pss = [emit_scores(cbp, qsl, 0), emit_scores(cbp, qsl, 1)]
                    prev = None
                    for kt in range(NT):
                        ps = pss.pop(0)
                        pt = ptpool.tile([128, 2, 512], BF16, tag="pt")
                        if kt in DVE_KTS:
                            nc.vector.tensor_scalar(
                                pt.bitcast(I16), ps, A16, B16, op0=MUL, op1=ADD)
                        else:
                            nc.scalar.activation(pt, ps, AF.Exp, scale=0.125)
                        if prev is not None:
                            ppt, pkt = prev
                            nc.tensor.matmul(
                                po[:, 0, :], vaug[:, pkt, 2 * cbp, :],
                                ppt[:, 0, :], start=(pkt == 0), stop=False,
                            )
                        if kt + 2 < NT:
                            pss.append(emit_scores(cbp, qsl, kt + 2))
                        if prev is not None:
                            nc.tensor.matmul(
                                po[:, 1, :], vaug[:, pkt, 2 * cbp + 1, :],
                                ppt[:, 1, :], start=(pkt == 0), stop=False,
                            )
                        prev = (pt, kt)
                        if proj_pend and kt in (5, 11):
                            emit_proj_tile(proj_pend.pop(0))
                    ppt, pkt = prev
                    for hh in range(2):
                        nc.tensor.matmul(
                            po[:, hh, :], vaug[:, pkt, 2 * cbp + hh, :],
                            ppt[:, hh, :], start=False, stop=True,
                        )
                    # normalize: aT rows of head = out rows * recip(denom rows)
                    rec = rpool.tile([128, 2, 512], F32, tag="rec")
                    nc.vector.reciprocal_approx_fast(rec, po)
                    # even head: out rows 0:64, denom rows 64:128; odd: swapped
                    nc.vector.tensor_mul(
                        aT[0:64, cbp, qsl], po[0:64, 0, :], rec[64:128, 0, :])
                    nc.vector.tensor_mul(
                        aT[64:128, cbp, qsl], po[64:128, 1, :], rec[0:64, 1, :])
                proj_pend.extend(range(qh4 * 4, (qh4 + 1) * 4))
            for t in proj_pend:
                emit_proj_tile(t)

    nc.finalize()
    return nc


def make_in_maps(x, qkv_w, qkv_b, q_norm_w, k_norm_w, proj_w, proj_b):
    """Shard the full inputs into the 8 per-core input maps."""
    bf = ml_dtypes.bfloat16
    qw = np.tile(q_norm_w.astype(np.float64), HPC)      # [256]
    kw = np.tile(k_norm_w.astype(np.float64), HPC)
    in_maps = []
    for c in range(8):
        b, g = c // 4, c % 4
        ch = np.arange(4 * g * HD, 4 * (g + 1) * HD)    # this core's head channels
        # columns: q (w-folded) | k (w-folded) | v
        wq = qkv_w[:, ch] * qw[None, :]
        wk = qkv_w[:, C + ch] * kw[None, :]
        wv = qkv_w[:, 2 * C + ch]
        wqkv_c = np.concatenate([wq, wk, wv], axis=1)
        bqk = np.concatenate([qkv_b[ch] * qw, qkv_b[C + ch] * kw])  # [512]
        bv = qkv_b[2 * C + ch]
        # block-diag head-incidence with 1/(64 w^2): [p, {q,k}, p']
        inc = np.zeros((128, 2, 128), np.float64)
        blk = (np.arange(128)[:, None] // HD) == (np.arange(128)[None, :] // HD)
        inc[:, 0, :] = blk / (64.0 * np.tile(q_norm_w.astype(np.float64), 2)[:, None] ** 2)
        inc[:, 1, :] = blk / (64.0 * np.tile(k_norm_w.astype(np.float64), 2)[:, None] ** 2)
        # wproj rows for this core as [128 rows of head-pair, pair, C]
        wproj_c = proj_w[ch, :].reshape(2, V // 2, C).transpose(1, 0, 2)
        in_maps.append({
            "x": np.ascontiguousarray(x[b].T).astype(bf),
            "wqkv": np.ascontiguousarray(wqkv_c).astype(bf),
            "bqk": np.ascontiguousarray(bqk.reshape(4, 128).T, np.float32),
            "bv": np.ascontiguousarray(bv, np.float32),
            "inc": np.ascontiguousarray(inc).astype(bf),
            "wproj": np.ascontiguousarray(wproj_c).astype(bf),
        })
    return in_maps


_NC_CACHE = []


def kernel(x, qkv_w, qkv_b, q_norm_w, k_norm_w, proj_w, proj_b,
           _run_kwargs=None, _res_box=None):
    x = np.asarray(x); qkv_w = np.asarray(qkv_w); qkv_b = np.asarray(qkv_b)
    q_norm_w = np.asarray(q_norm_w); k_norm_w = np.asarray(k_norm_w)
    proj_w = np.asarray(proj_w); proj_b = np.asarray(proj_b)

    if not _NC_CACHE:
        _NC_CACHE.append(build_nc())
    nc = _NC_CACHE[0]
    in_maps = make_in_maps(x, qkv_w, qkv_b, q_norm_w, k_norm_w, proj_w, proj_b)
    res = run_bass_kernel_spmd(nc, in_maps, core_ids=list(range(8)),
                               **(_run_kwargs or {}))
    if _res_box is not None:
        _res_box["res"] = res
    out = np.zeros((B, N, C), np.float32)
    for c in range(8):
        out[c // 4] += res.results[c]["out"]
    out += proj_b[None, None, :].astype(np.float32)
    return out


if __name__ == "__main__":
    rng = np.random.default_rng(0)
    x = rng.standard_normal((B, N, C)).astype(np.float32)
    qkv_w = (rng.standard_normal((C, 3 * C)) / np.sqrt(C)).astype(np.float32)
    qkv_b = np.zeros((3 * C,), np.float32)
    qn_w = np.ones((HD,), np.float32)
    kn_w = np.ones((HD,), np.float32)
    proj_w = (rng.standard_normal((C, C)) / np.sqrt(C)).astype(np.float32)
    proj_b = np.zeros((C,), np.float32)
    out = kernel(x, qkv_w, qkv_b, qn_w, kn_w, proj_w, proj_b)
    print("out", out.shape, out.dtype, float(np.abs(out).mean()))
